# revision 1
# baseline (speedup 1.0000x reference)
"""Trainium2 Bass kernel for C = tril(A @ B), A/B lower-triangular 4096x4096 fp32.

Mod-2 column-interleaved col split, fully SBUF-resident inputs, fp16
output, modest p-state gap fillers, small-last-eviction tail.  Measured
71.3-73us on HW (machine-state dependent, +-2us) at 3.6e-4 rel err;
session baseline was 77.9us.

Distribution (SPMD, 8 cores = 4 row-groups x 2 col-groups): core (g, h) owns
row-blocks {4t+g : t=0..7} (slots) and 128-col-blocks {2u+h : u=0..15},
grouped into 4 free-dim groups l=0..3 of 4 col-blocks {8l+2v+h : v=0..3}.
Slot t uses uniform K bound of 4t+4 k-blocks; group l covers k-blocks
8l..31 with a pair-taper: k-pair j of a group has free width
w_j = 128*min(4, j+1) (the mod-2 interleave makes both k-planes of a pair
share the same width). Inputs exactly triangular => over-computed terms are
exact zeros.

Modes: float16 (default) / bfloat16; fp8x3/fp8u3 (float8e4 DoubleRow
3-term compensation) are numerically fine (~1.3e-3) but ~1.5x slower on
real HW (DoubleRow weight loads do not pipeline with exec).
Timeline: ~10us launch+DMA-queue-ramp (runtime-fixed) + ~54us PE window
(113,664 cycles @2.4GHz, the SPMD identical-shapes optimum for this
grid; PE clock ramps 0.65->1.2->2.4GHz and resets on ~0.3us idle, so
the DMA-paced early slots run at mid clock) + ~5.5-6.5us tail (last
eviction chain + ~3.2us NEFF drain).

Per-core DMA: A pack 4.72MB + B pack 8.91MB + fp16 out 2.36MB = 15.99MB.
"""

import numpy as np

N = 4096
P = 128
NCORES = 8
RG, CG = 4, 2
SLOTS = 8            # row-block slots per core
L = 4                # col groups per core
KB = 32              # k-blocks

MM_DT_NAME = "float16"

KEND = [4 * t + 4 for t in range(SLOTS)]           # k-blocks per slot
A_OFF = [sum(KEND[:t]) for t in range(SLOTS)]
A_TOT = sum(KEND)                                  # 144 k-blocks

NPAIR = [16 - 4 * l for l in range(L)]             # k-pairs per group
PAIRW = {l: [128 * min(4, j + 1) for j in range(NPAIR[l])] for l in range(L)}
# B stream order: l-major, j ascending.  Offsets in "w units" (cols of one
# k-plane); the dram/sbuf layouts scale by planes-per-pair (2 fp16 / 4 fp8).
B_OFF = {}
_off = 0
for _l in range(L):
    for _j in range(NPAIR[_l]):
        B_OFF[(_l, _j)] = _off
        _off += PAIRW[_l][_j]
B_TOTW = _off                                      # 17408 w-units

ACTIVE = [(t, l) for l in range(L) for t in range(SLOTS) if KEND[t] - 8 * l > 0]


def _m(t, l):
    return KEND[t] - 8 * l


def _wout(t, l):
    return min(512, 64 * _m(t, l))


_cached = {}


def _build(mm_dt_name):
    import concourse.mybir as mybir
    import concourse.tile as tile
    from concourse import bacc

    fp8 = mm_dt_name == "fp8x3"
    fp8u = mm_dt_name in ("fp8u3", "fp8u2")   # unfused packs, contiguous lhsT
    terms3 = mm_dt_name in ("fp8x3", "fp8u3")
    if fp8 or fp8u:
        mm_dt = mybir.dt.float8e4
        apl, bpl = (2, 4) if fp8 else (1, 2)
        dr = mybir.MatmulPerfMode.DoubleRow
    else:
        mm_dt = getattr(mybir.dt, mm_dt_name)
        apl, bpl = 1, 2

    nc = bacc.Bacc("TRN2", target_bir_lowering=False, debug=False,
                   num_devices=NCORES)
    at_d = nc.dram_tensor("at", [P, A_TOT * apl * P], mm_dt,
                          kind="ExternalInput").ap()
    b_d = nc.dram_tensor("b", [P, B_TOTW * bpl], mm_dt,
                         kind="ExternalInput").ap()
    at2_d = b2_d = None
    if fp8u:
        at2_d = nc.dram_tensor("at2", [P, A_TOT * P], mm_dt,
                               kind="ExternalInput").ap()
        if terms3:
            b2_d = nc.dram_tensor("b2", [P, B_TOTW * 2], mm_dt,
                                  kind="ExternalInput").ap()
    o_d = nc.dram_tensor("o", [SLOTS, P, L * 512], mybir.dt.float16,
                         kind="ExternalOutput").ap()

    with tile.TileContext(nc) as tc:
        with (
            tc.tile_pool(name="atp", bufs=1) as atp,
            tc.tile_pool(name="bp", bufs=1) as bp,
            tc.tile_pool(name="pp", bufs=1, space="PSUM") as pp,
            tc.tile_pool(name="sp", bufs=4) as sp,
        ):
            at_sb = {}
            at2_sb = {}
            bt = {}
            bt2 = {}

            def load_at(t):
                a = atp.tile([P, KEND[t], apl, P] if fp8 else [P, KEND[t], P],
                             mm_dt, tag=f"at{t}", name=f"at{t}")
                nc.sync.dma_start(
                    a[:], at_d[:, A_OFF[t] * apl * P:
                               (A_OFF[t] + KEND[t]) * apl * P])
                at_sb[t] = a
                if fp8u:
                    a2 = atp.tile([P, KEND[t], P], mm_dt, tag=f"at2_{t}",
                                  name=f"at2_{t}")
                    nc.sync.dma_start(
                        a2[:], at2_d[:, A_OFF[t] * P:(A_OFF[t] + KEND[t]) * P])
                    at2_sb[t] = a2

            def load_b(l, j):
                w = PAIRW[l][j]
                b = bp.tile([P, bpl, w], mm_dt, tag=f"b{l}_{j}",
                            name=f"b{l}_{j}")
                o0 = B_OFF[(l, j)] * bpl
                nc.sync.dma_start(b[:], b_d[:, o0:o0 + bpl * w])
                bt[(l, j)] = b
                if fp8u and terms3:
                    b2 = bp.tile([P, 2, w], mm_dt, tag=f"b2_{l}_{j}",
                                 name=f"b2_{l}_{j}")
                    o2 = B_OFF[(l, j)] * 2
                    nc.sync.dma_start(b2[:], b2_d[:, o2:o2 + 2 * w])
                    bt2[(l, j)] = b2

            def mm_pair(ps, t, l, j, start, stop):
                w = PAIRW[l][j]
                k = 8 * l + 2 * j          # absolute k-block of plane 0
                b = bt[(l, j)]
                a = at_sb[t]
                if fp8:
                    a1 = a[:, k:k + 2, 0, :]
                    a2 = a[:, k:k + 2, 1, :]
                    b1 = b[:, 0:2, :]
                    b2 = b[:, 2:4, :]
                    nc.tensor.matmul(ps[:, :w], lhsT=a1, rhs=b1,
                                     start=start, stop=False, perf_mode=dr)
                    nc.tensor.matmul(ps[:, :w], lhsT=a1, rhs=b2,
                                     start=False, stop=False, perf_mode=dr)
                    nc.tensor.matmul(ps[:, :w], lhsT=a2, rhs=b1,
                                     start=False, stop=stop, perf_mode=dr)
                elif fp8u:
                    a1 = a[:, k:k + 2, :]
                    a2 = at2_sb[t][:, k:k + 2, :]
                    b1 = b[:]
                    nc.tensor.matmul(ps[:, :w], lhsT=a1, rhs=b1,
                                     start=start, stop=False, perf_mode=dr)
                    if terms3:
                        nc.tensor.matmul(ps[:, :w], lhsT=a1,
                                         rhs=bt2[(l, j)][:],
                                         start=False, stop=False, perf_mode=dr)
                    nc.tensor.matmul(ps[:, :w], lhsT=a2, rhs=b1,
                                     start=False, stop=stop, perf_mode=dr)
                else:
                    for q in range(2):
                        nc.tensor.matmul(
                            ps[:, :w], lhsT=a[:, k + q, :],
                            rhs=b[:, q, :],
                            start=start and q == 0, stop=stop and q == 1)

            def evict(t, l, ps, split=False):
                w = _wout(t, l)
                st = sp.tile([P, 512], mybir.dt.float16, tag="st",
                             name=f"st{t}_{l}")
                nc.vector.tensor_copy(st[:, :w], ps[:, :w])
                # final pass rides the sync ring: the HWDGE trigger is much
                # cheaper than gpsimd's ~650ns DIRECT2D trigger, which
                # dominates the closing eviction chain (trace-verified)
                eng = nc.sync if l == L - 1 else nc.gpsimd
                eng.dma_start(o_d[t, :, l * 512:l * 512 + w], st[:, :w])

            # ---- pass 0 (l=0): slot-major ----
            for t in range(SLOTS):
                load_at(t)
                load_b(0, 2 * t)
                load_b(0, 2 * t + 1)
                ps = pp.tile([P, 512], mybir.dt.float32, tag=f"bk{t}",
                             name=f"ps{t}_0")
                np0 = _m(t, 0) // 2
                for j in range(np0):
                    mm_pair(ps, t, 0, j, j == 0, j == np0 - 1)
                evict(t, 0, ps)
                # modest p-state gap fillers: keep the PE ticking through
                # the DMA-paced early slots (measured ~0.5us net win; bigger
                # fill schemes lose -- the ramp resets on any ~0.3us idle)
                if t <= 6:
                    scr = pp.tile([P, 512], mybir.dt.float32,
                                  tag="bk7", name=f"scr{t}")
                    for d in range(6):
                        nc.tensor.matmul(scr[:, :P], lhsT=at_sb[0][:, 0, :],
                                         rhs=bt[(0, 0)][:, 0, :],
                                         start=True, stop=True)

            # ---- passes l=1,2: k-pair-major ----
            for l in (1, 2):
                psums = {}
                if l == 1:
                    scr = pp.tile([P, 512], mybir.dt.float32,
                                  tag="bk7", name="scr_p1")
                    for d in range(4):
                        nc.tensor.matmul(scr[:, :P], lhsT=at_sb[0][:, 0, :],
                                         rhs=bt[(0, 0)][:, 0, :],
                                         start=True, stop=True)

                for j in range(NPAIR[l]):
                    load_b(l, j)
                    for t in range(2 * l, SLOTS):
                        npt = _m(t, l) // 2
                        if j >= npt:
                            continue
                        if j == 0:
                            # remapped banks: free for >=10us at alloc time,
                            # so no wait on the previous pass's eviction cast
                            bk = t - 2 if l == 1 else {4: 6, 5: 7, 6: 0,
                                                       7: 1}[t]
                            psums[t] = pp.tile([P, 512], mybir.dt.float32,
                                               tag=f"bk{bk}", name=f"ps{t}_{l}")
                        mm_pair(psums[t], t, l, j, j == 0, j == npt - 1)
                        if j == npt - 1:
                            evict(t, l, psums[t])

            # ---- pass l=3 (last): slot-major with slot 7 first, so the
            # kernel ends on slot 6's small (w=256) eviction; slot 6 pair 0
            # leads to cover slot 7's PSUM-bank wait on its pass-2 evict ----
            for j in range(NPAIR[3]):
                load_b(3, j)
            ps6 = pp.tile([P, 512], mybir.dt.float32, tag="bk2", name="ps6_3")
            ps7 = pp.tile([P, 512], mybir.dt.float32, tag="bk3", name="ps7_3")
            mm_pair(ps6, 6, 3, 0, True, False)
            for j in range(4):
                mm_pair(ps7, 7, 3, j, j == 0, j == 3)
            mm_pair(ps6, 6, 3, 1, False, True)
            # evict t6 FIRST: the DVE runs casts in emission order, and
            # t6's stop lands before t7's (readiness-scheduled), so its
            # cast overlaps t7's last matmuls instead of queueing after
            # t7's cast at the very end (trace-verified ordering)
            evict(6, 3, ps6)
            evict(7, 3, ps7)

    nc.compile()
    return nc


def _get_nc(mm_dt_name):
    if mm_dt_name not in _cached:
        _cached[mm_dt_name] = _build(mm_dt_name)
    return _cached[mm_dt_name]


def _np_dt(mm_dt_name):
    import ml_dtypes
    if mm_dt_name == "float16":
        return np.float16
    if mm_dt_name == "bfloat16":
        return ml_dtypes.bfloat16
    if mm_dt_name in ("fp8x3", "fp8u3", "fp8u2"):
        return ml_dtypes.float8_e4m3fn
    return np.float32


def _pack_at(A, g, np_dt):
    """fp16: [P, 144*128], slot t planes = A^T k-blocks 0..4t+4 of row 4t+g."""
    out = np.empty((P, A_TOT * P), dtype=np_dt)
    for t in range(SLOTS):
        blk = RG * t + g
        E = KEND[t] * P
        blockT = A[blk * P:(blk + 1) * P, :E].T.astype(np_dt)   # [kk, m]
        arr = blockT.reshape(KEND[t], P, P).transpose(1, 0, 2)
        out[:, A_OFF[t] * P:(A_OFF[t] + KEND[t]) * P] = \
            arr.reshape(P, KEND[t] * P)
    return out


def _pack_at8(A, g, np_dt):
    """fp8: [P, 144*2*128], per k-block planes (A1, A2) interleaved."""
    out = np.empty((P, A_TOT * 2 * P), dtype=np_dt)
    for t in range(SLOTS):
        blk = RG * t + g
        E = KEND[t] * P
        blockT = A[blk * P:(blk + 1) * P, :E].T                 # [kk, m] f32
        a1 = blockT.astype(np_dt)
        a2 = (blockT - a1.astype(np.float32)).astype(np_dt)
        arr = np.stack([a1.reshape(KEND[t], P, P),
                        a2.reshape(KEND[t], P, P)], axis=1)     # [nk, 2, kk, m]
        arr = arr.transpose(2, 0, 1, 3)                         # [kk, nk, 2, m]
        out[:, A_OFF[t] * 2 * P:(A_OFF[t] + KEND[t]) * 2 * P] = \
            arr.reshape(P, KEND[t] * 2 * P)
    return out


def _pack_b(B, h, np_dt, fp8):
    """[P, B_TOTW*bpl]: pair (l,j) = [kk, bpl, w]; planes (B(k0),B(k1)) fp16
    or (B1(k0),B1(k1),B2(k0),B2(k1)) fp8; cols = col-blocks {8l+2v+h}."""
    bpl = 4 if fp8 else 2
    B4 = B.reshape(KB, P, KB, P)
    out = np.empty((P, B_TOTW * bpl), dtype=np_dt)
    for l in range(L):
        for j in range(NPAIR[l]):
            w = PAIRW[l][j]
            nv = w // 128
            cols = [8 * l + 2 * v + h for v in range(nv)]
            pair = np.empty((P, bpl, w), dtype=np_dt)
            for q in range(2):
                k = 8 * l + 2 * j + q
                plane = np.concatenate([B4[k, :, c, :] for c in cols],
                                       axis=1)                  # [kk, w] f32
                if fp8:
                    p1 = plane.astype(np_dt)
                    pair[:, q, :] = p1
                    pair[:, 2 + q, :] = (
                        plane - p1.astype(np.float32)).astype(np_dt)
                else:
                    pair[:, q, :] = plane.astype(np_dt)
            o0 = B_OFF[(l, j)] * bpl
            out[:, o0:o0 + bpl * w] = pair.reshape(P, bpl * w)
    return out


def kernel(A, B, mm_dt_name=MM_DT_NAME, trace=False):
    from concourse.bass_utils import run_bass_kernel_spmd

    A = np.ascontiguousarray(np.asarray(A, dtype=np.float32))
    B = np.ascontiguousarray(np.asarray(B, dtype=np.float32))

    nc = _get_nc(mm_dt_name)
    np_dt = _np_dt(mm_dt_name)
    fp8 = mm_dt_name == "fp8x3"
    fp8u = mm_dt_name in ("fp8u3", "fp8u2")
    terms3 = mm_dt_name in ("fp8x3", "fp8u3")
    pa = _pack_at8 if fp8 else _pack_at
    b_packs = [_pack_b(B, h, np_dt, fp8) for h in range(CG)]
    in_maps = [{"at": pa(A, c % RG, np_dt), "b": b_packs[c // RG]}
               for c in range(NCORES)]
    if fp8u:
        Alo = A - A.astype(np_dt).astype(np.float32)
        Blo = B - B.astype(np_dt).astype(np.float32)
        at2_packs = [_pack_at(Alo, g, np_dt) for g in range(RG)]
        b2_packs = [_pack_b(Blo, h, np_dt, False) for h in range(CG)] \
            if terms3 else None
        for c in range(NCORES):
            in_maps[c]["at2"] = at2_packs[c % RG]
            if terms3:
                in_maps[c]["b2"] = b2_packs[c // RG]

    res = None
    for attempt in range(3):
        try:
            res = run_bass_kernel_spmd(nc, in_maps,
                                       core_ids=list(range(NCORES)),
                                       trace=trace)
            break
        except Exception:
            if attempt == 2:
                raise
            import time
            time.sleep(2)
    C = np.zeros((N, N), dtype=np.float32)
    for c in range(NCORES):
        g, h = c % RG, c // RG
        o = res.results[c]["o"]
        for t in range(SLOTS):
            blk = RG * t + g
            for l in range(L):
                if _m(t, l) <= 0:
                    continue
                for v in range(_wout(t, l) // 128):
                    cb = 8 * l + 2 * v + h
                    C[blk * P:(blk + 1) * P, cb * P:(cb + 1) * P] = \
                        o[t, :, l * 512 + v * 128:l * 512 + (v + 1) * 128]
    if trace:
        kernel.last_exec_time_ns = res.exec_time_ns
        kernel.last_results = res
    return C

